# revision 23
# baseline (speedup 1.0000x reference)
"""Trainium2 Bass kernel for nn_AttentionWriter (scatter_memory).

Data-parallel over batch: 8 batch elements -> 8 NeuronCores, one each.
Per-core pipeline (S=1024 tokens, M=2048 slots, D=H=512, 8 heads of 64):
  - transpose inputs to feature-major via TensorE
  - fused projections with float32r matmuls (1 cyc/row)
  - cross-attention with ACT-engine exp, softmax denominators via
    DVE tree-add + ones-matmul broadcast, bf16 AV matmuls
  - argmax/threshold logic done branch-free with iota + compare masks
  - scatter of the updated row via a rank-1 (one-hot x delta) matmul
"""

import sys

sys.path.insert(0, "/opt/trn_rl_repo")

from contextlib import ExitStack

import ml_dtypes
import numpy as np

import concourse.bass as bass
import concourse.mybir as mybir
from concourse import bacc, bass_isa
from concourse.bass_utils import run_bass_kernel_spmd
from concourse.masks import make_identity
from concourse.tile import TileContext

F32 = mybir.dt.float32
F32R = mybir.dt.float32r
BF16 = mybir.dt.bfloat16
I32 = mybir.dt.int32
AF = mybir.ActivationFunctionType
OP = mybir.AluOpType
AX = mybir.AxisListType

import os

DEBUG = bool(int(os.environ.get("KDEBUG", "0")))
B, S, M, D, H, NH = 8, 1024, 2048, 512, 512, 8
DH = H // NH  # 64
P = 128
SC, MC, DC, HC = S // P, M // P, D // P, H // P  # 8, 16, 4, 4

PARAM_SPECS = {
    "W_in": (D, H), "b_in": (H,),
    "Wq": (H, H), "bq": (H,), "Wk": (H, H), "bk": (H,),
    "Wv": (H, H), "bv": (H,), "Wo": (H, H), "bo": (H,),
    "Wi1": (H, H // 2), "bi1": (H // 2,), "Wi2": (H // 2, 1), "bi2": (1,),
    "Wu1": (2 * H, H), "bu1": (H,), "Wu2": (H, H), "bu2": (H,),
}
MAT_PARAMS = {"W_in", "Wq", "Wk", "Wv", "Wo", "Wi1", "Wi2", "Wu1", "Wu2"}


def r(ap):
    return ap


def build():
    nc = bacc.Bacc(None, target_bir_lowering=False)

    ni_ext = nc.declare_dram_parameter("new_info", [S, D], F32, isOutput=False)
    mb_ext = nc.declare_dram_parameter("memory_bank", [M, D], F32, isOutput=False)
    pext = {
        k: nc.declare_dram_parameter(k, list(v),
                                     BF16 if k in MAT_PARAMS else F32,
                                     isOutput=False)
        for k, v in PARAM_SPECS.items()
    }
    upd_ext = nc.declare_dram_parameter("updated", [M, D], F32, isOutput=True)
    ww_ext = nc.declare_dram_parameter("write_weights", [M, H], F32, isOutput=True)
    imp_ext = nc.declare_dram_parameter("imp", [S], F32, isOutput=True)

    with TileContext(nc) as tc, ExitStack() as ctx:
        pp = ctx.enter_context(tc.tile_pool(name="persist", bufs=1))
        pst = ctx.enter_context(tc.tile_pool(name="stats", bufs=1))

        ident = pp.tile([P, P], F32, tag="ident")
        make_identity(nc, ident)
        ident_bf = pp.tile([P, P], BF16, tag="ident_bf")
        nc.vector.tensor_copy(ident_bf, ident)
        zero512 = pp.tile([P, 512], F32, tag="zero512")
        nc.gpsimd.memset(zero512, 0.0)
        ones128 = pp.tile([P, P], BF16, tag="ones128")
        nc.vector.tensor_scalar(ones128, zero512[:, 0:P], 1.0, None, OP.add)

        ww_tm = pp.tile([P, MC, H], F32, tag="ww_tm")   # write_weights tok-major
        rs_cols = pp.tile([P, MC], F32, tag="rs_cols")  # row sums of ww
        imp_row = pp.tile([1, S], F32, tag="imp_row")

        s12 = ctx.enter_context(ExitStack())
        qkv = s12.enter_context(tc.tile_pool(name="qkv", bufs=1))
        q_T = qkv.tile([P, HC, M], BF16, tag="q_T")
        k_T = qkv.tile([P, HC, S], BF16, tag="k_T")
        v_tm = qkv.tile([P, SC, H], BF16, tag="v_tm")
        wo_s = qkv.tile([P, HC, H], BF16, tag="wo_s")
        bo_b = qkv.tile([P, H], F32, tag="bo_b")

        # ============ stage 0/1: transposes + projections ============
        with ExitStack() as s1:
            p1 = s1.enter_context(tc.tile_pool(name="ph1", bufs=1))
            pstr = s1.enter_context(tc.tile_pool(name="stripes", bufs=2))
            ps_tp = s1.enter_context(tc.tile_pool(name="ps_tp", bufs=4,
                                                  space="PSUM"))
            ps_pj = s1.enter_context(tc.tile_pool(name="ps_pj", bufs=4,
                                                  space="PSUM"))

            w_in = p1.tile([P, DC, H], BF16, tag="w_in")
            nc.sync.dma_start(w_in[:], pext["W_in"].rearrange("(c p) h -> p c h", p=P))
            b_in_c = pst.tile([P, HC], F32, tag="b_in_c")
            nc.sync.dma_start(b_in_c[:], pext["b_in"].rearrange("(c p) -> p c", p=P))

            ip_T = p1.tile([P, HC, S], BF16, tag="ip_T")  # info_proj [h, s]

            def proj_in(src_ext, nchunks, out_T):
                # per 512-col stripe: DMA token-major, transpose, project
                for fb in range(nchunks * P // 512):  # 512-wide output stripes
                    xT = pstr.tile([P, DC, 512], BF16, tag="xT", name="xT")
                    for q in range(4):  # four 128-row blocks per stripe
                        rb = fb * 4 + q
                        blk = pstr.tile([P, D], F32, tag="blk", name="blk")
                        nc.sync.dma_start(blk, src_ext[rb * P:(rb + 1) * P, :])
                        blk2 = pstr.tile([P, D], BF16, tag="blk2", name="blk2")
                        nc.vector.tensor_copy(blk2, blk)
                        for dc in range(DC):
                            tps = ps_tp.tile([P, P], BF16, tag="tp", name="tp")
                            nc.tensor.transpose(
                                tps, blk2[:, dc * P:(dc + 1) * P], ident_bf)
                            nc.vector.tensor_copy(
                                xT[:, dc, q * P:(q + 1) * P], tps)
                    for hc in range(HC):
                        ps = ps_pj.tile([P, 512], F32, tag="pj", name="pj")
                        for dc in range(DC):
                            nc.tensor.matmul(
                                ps, r(w_in[:, dc, hc * P:(hc + 1) * P]),
                                r(xT[:, dc, :]),
                                start=(dc == 0), stop=(dc == DC - 1))
                        nc.scalar.activation(
                            out_T[:, hc, fb * 512:(fb + 1) * 512], ps,
                            AF.Identity, bias=b_in_c[:, hc:hc + 1])

            proj_in(ni_ext, SC, ip_T)

            with ExitStack() as s1m:
                p1m = s1m.enter_context(tc.tile_pool(name="ph1m", bufs=1))
                mp_T = p1m.tile([P, HC, M], BF16, tag="mp_T")
                proj_in(mb_ext, MC, mp_T)

                # Q projection (uses mp_T, then it dies)
                wq = p1m.tile([P, HC, H], BF16, tag="wq")
                nc.sync.dma_start(wq[:],
                                  pext["Wq"].rearrange("(c p) h -> p c h", p=P))
                bq_c = pst.tile([P, HC], F32, tag="bq_c")
                nc.sync.dma_start(bq_c[:],
                                  pext["bq"].rearrange("(c p) -> p c", p=P))
                for hc in range(HC):
                    for mf in range(M // 512):
                        ps = ps_pj.tile([P, 512], F32, tag="pj", name="pj")
                        for kc in range(HC):
                            nc.tensor.matmul(
                                ps, r(wq[:, kc, hc * P:(hc + 1) * P]),
                                r(mp_T[:, kc, mf * 512:(mf + 1) * 512]),
                                start=(kc == 0), stop=(kc == HC - 1))
                        nc.scalar.activation(
                            q_T[:, hc, mf * 512:(mf + 1) * 512], ps, AF.Identity,
                            bias=bq_c[:, hc:hc + 1])

            s1.enter_context(tc.tile_pool(name="stripes_close", bufs=1))
            p1c = s1.enter_context(tc.tile_pool(name="ph1c", bufs=1))
            # K projection
            wk = p1c.tile([P, HC, H], BF16, tag="wsh", name="wk")
            nc.sync.dma_start(wk[:], pext["Wk"].rearrange("(c p) h -> p c h", p=P))
            bk_c = pst.tile([P, HC], F32, tag="bk_c")
            nc.sync.dma_start(bk_c[:], pext["bk"].rearrange("(c p) -> p c", p=P))
            for hc in range(HC):
                for sf in range(S // 512):
                    ps = ps_pj.tile([P, 512], F32, tag="pj", name="pj")
                    for kc in range(HC):
                        nc.tensor.matmul(
                            ps, r(wk[:, kc, hc * P:(hc + 1) * P]),
                            r(ip_T[:, kc, sf * 512:(sf + 1) * 512]),
                            start=(kc == 0), stop=(kc == HC - 1))
                    nc.scalar.activation(
                        k_T[:, hc, sf * 512:(sf + 1) * 512], ps, AF.Identity,
                        bias=bk_c[:, hc:hc + 1])

            # V projection, token-major [s, h'], bf16
            wv = p1c.tile([P, HC, H], BF16, tag="wsh", name="wv")
            nc.sync.dma_start(wv[:], pext["Wv"].rearrange("(c p) h -> p c h", p=P))
            bv_row = pst.tile([1, H], F32, tag="bv_row")
            nc.sync.dma_start(bv_row[:], pext["bv"].rearrange("(o h) -> o h", o=1))
            bv_b = p1c.tile([P, H], F32, tag="bv_b")
            nc.gpsimd.partition_broadcast(bv_b, bv_row)
            for st in range(SC):
                ps = ps_pj.tile([P, 512], F32, tag="pj", name="pj")
                for hc in range(HC):
                    nc.tensor.matmul(
                        ps, r(ip_T[:, hc, st * P:(st + 1) * P]),
                        r(wv[:, hc, :]),
                        start=(hc == 0), stop=(hc == HC - 1))
                nc.vector.tensor_tensor(v_tm[:, st, :], ps, bv_b, OP.add)

            # importance MLP: imp = sigmoid(relu(ip @ Wi1 + bi1) @ Wi2 + bi2)
            wi1 = p1c.tile([P, HC, H // 2], BF16, tag="wi1")
            nc.sync.dma_start(wi1[:], pext["Wi1"].rearrange("(c p) j -> p c j", p=P))
            bi1_c = pst.tile([P, 2], F32, tag="bi1_c")
            nc.sync.dma_start(bi1_c[:], pext["bi1"].rearrange("(c p) -> p c", p=P))
            wi2 = p1c.tile([P, 2, 1], BF16, tag="wi2")
            nc.sync.dma_start(wi2[:], pext["Wi2"].rearrange("(c p) o -> p c o", p=P))
            bi2_t = pst.tile([1, 1], F32, tag="bi2_t")
            nc.sync.dma_start(bi2_t[:], pext["bi2"].rearrange("(o s) -> o s", o=1))
            bi2_h = pst.tile([1, 1], F32, tag="bi2_h")
            nc.vector.tensor_scalar(bi2_h, bi2_t, 0.5, None, OP.mult)

            h1_T = p1c.tile([P, 2, S], BF16, tag="h1_T")
            for jc in range(2):
                for sf in range(S // 512):
                    ps = ps_pj.tile([P, 512], F32, tag="pj", name="pj")
                    for hc in range(HC):
                        nc.tensor.matmul(
                            ps, r(wi1[:, hc, jc * P:(jc + 1) * P]),
                            r(ip_T[:, hc, sf * 512:(sf + 1) * 512]),
                            start=(hc == 0), stop=(hc == HC - 1))
                    nc.scalar.activation(
                        h1_T[:, jc, sf * 512:(sf + 1) * 512], ps, AF.Relu,
                        bias=bi1_c[:, jc:jc + 1])
            for sf in range(S // 512):
                ps = ps_pj.tile([P, 512], F32, tag="pj", name="pj")
                for jc in range(2):
                    nc.tensor.matmul(
                        ps[0:1, :], r(wi2[:, jc, :]),
                        r(h1_T[:, jc, sf * 512:(sf + 1) * 512]),
                        start=(jc == 0), stop=(jc == 1))
                # sigmoid(x) = 0.5 + 0.5*tanh(x/2); keeps Exp+Tanh in one
                # ACT table set (no set holds both Sigmoid and Exp)
                nc.scalar.activation(
                    imp_row[0:1, sf * 512:(sf + 1) * 512], ps[0:1, :], AF.Tanh,
                    bias=bi2_h[0:1, 0:1], scale=0.5)
                nc.vector.tensor_scalar(
                    imp_row[0:1, sf * 512:(sf + 1) * 512],
                    imp_row[0:1, sf * 512:(sf + 1) * 512], 0.5, 0.5,
                    OP.mult, OP.add)
            nc.sync.dma_start(imp_ext.rearrange("(o s) -> o s", o=1), imp_row)

            # mean importance -> fold into Wo
            mi = pst.tile([1, 1], F32, tag="mi")
            nc.vector.tensor_reduce(mi, imp_row, AX.X, OP.add)
            nc.vector.tensor_scalar(mi, mi, 1.0 / S, None, OP.mult)
            mi_b = pst.tile([P, 1], F32, tag="mi_b")
            nc.gpsimd.partition_broadcast(mi_b, mi)

            wo = p1c.tile([P, HC, H], BF16, tag="wsh", name="wo")
            nc.sync.dma_start(wo[:], pext["Wo"].rearrange("(c p) h -> p c h", p=P))
            for hc in range(HC):
                nc.vector.tensor_scalar(
                    wo_s[:, hc, :], wo[:, hc, :], mi_b[:, 0:1], None, OP.mult)
            bo_row = pst.tile([1, H], F32, tag="bo_row")
            nc.sync.dma_start(bo_row[:], pext["bo"].rearrange("(o h) -> o h", o=1))
            bo_mi = pst.tile([1, H], F32, tag="bo_mi")
            nc.vector.tensor_scalar(bo_mi, bo_row, mi[0:1, 0:1], None, OP.mult)
            nc.gpsimd.partition_broadcast(bo_b, bo_mi)

        # ============ stage 2: attention ============
        MS = 1024  # m-supertile
        with ExitStack() as s2:
            pa = s2.enter_context(tc.tile_pool(name="attn", bufs=1))
            ps_sc = s2.enter_context(tc.tile_pool(name="ps_sc", bufs=2,
                                                  space="PSUM"))
            ps_z = s2.enter_context(tc.tile_pool(name="ps_z", bufs=1,
                                                 space="PSUM"))
            ps_b = s2.enter_context(tc.tile_pool(name="ps_b", bufs=2,
                                                 space="PSUM"))
            for ms in range(M // MS):
                av_T = pa.tile([P, HC, MS], BF16, tag="av_T", name="av_T")
                for pc in range(HC):  # head pair (2pc, 2pc+1)
                    expp = [pa.tile([P, SC, MS], BF16, tag=f"exp{hi}",
                                    name=f"exp{hi}")
                            for hi in range(2)]
                    # scores^T + exp
                    for st in range(SC):
                        pss = [ps_sc.tile([P, MS], F32, tag="sc", name="sc")
                               for _ in range(2)]
                        for hi in range(2):
                            for mf in range(MS // 512):
                                nc.tensor.matmul(
                                    pss[hi][:, mf * 512:(mf + 1) * 512],
                                    k_T[hi * DH:(hi + 1) * DH, pc,
                                        st * P:(st + 1) * P],
                                    q_T[hi * DH:(hi + 1) * DH, pc,
                                        ms * MS + mf * 512:
                                        ms * MS + (mf + 1) * 512],
                                    start=True, stop=True,
                                    tile_position=(hi * DH, 0))
                        for hi in range(2):
                            nc.scalar.activation(
                                expp[hi][:, st, :], pss[hi], AF.Exp,
                                bias=0.0, scale=1.0 / np.sqrt(DH))
                    # softmax denominators: tree-add + ones-matmul broadcast
                    # (reciprocal_approx_fast misbehaves at base_partition!=0,
                    #  so each head gets a full-width recip tile)
                    recz = [pa.tile([P, MS], F32, tag=f"recz{hi}",
                                    name="recz") for hi in range(2)]
                    for hi in range(2):
                        t4 = pa.tile([P, 4, MS], BF16, tag="zt4", name="zt4")
                        nc.vector.tensor_tensor(
                            t4, expp[hi][:, 0:4, :], expp[hi][:, 4:8, :], OP.add)
                        t2 = pa.tile([P, 2, MS], BF16, tag="zt2", name="zt2")
                        nc.vector.tensor_tensor(
                            t2, t4[:, 0:2, :], t4[:, 2:4, :], OP.add)
                        z128 = pa.tile([P, MS], BF16, tag="z128", name="z128")
                        nc.vector.tensor_tensor(
                            z128, t2[:, 0, :], t2[:, 1, :], OP.add)
                        psz = ps_z.tile([P, MS], F32, tag="z", name="z")
                        for mf in range(MS // 512):
                            nc.tensor.matmul(
                                psz[:, mf * 512:(mf + 1) * 512], r(ones128),
                                r(z128[:, mf * 512:(mf + 1) * 512]),
                                start=True, stop=True)
                        nc.vector.reciprocal_approx_fast(recz[hi], psz)
                    if DEBUG and ms == 0 and pc == 0:
                        dbg = [(expp[0][:, 0, 0:512], 0), (t4[:, 0, 0:512], 1),
                               (z128[:, 0:512], 2), (recz[1][:, 0:512], 3),
                               (expp[1][:, 0, 0:512], 4)]
                        for ap, rowg in dbg:
                            dt_ = pa.tile([P, 512], F32, tag=f"dbg{rowg}",
                                          name="dbg")
                            nc.vector.tensor_copy(dt_, ap)
                            nc.sync.dma_start(
                                ww_ext[rowg * P:(rowg + 1) * P, :], dt_)
                    # AV (unnormalized) then normalize on copy-out
                    for mf in range(MS // 512):
                        psav = ps_b.tile([P, 512], F32, tag="pb", name="pb")
                        for hi in range(2):
                            h = 2 * pc + hi
                            for st in range(SC):
                                nc.tensor.matmul(
                                    psav[hi * DH:(hi + 1) * DH, :],
                                    v_tm[:, st, h * DH:(h + 1) * DH],
                                    expp[hi][:, st, mf * 512:(mf + 1) * 512],
                                    start=(st == 0), stop=(st == SC - 1),
                                    tile_position=(0, hi * DH))
                        for hi in range(2):
                            sl = slice(hi * DH, (hi + 1) * DH)
                            nc.vector.tensor_tensor(
                                av_T[sl, pc, mf * 512:(mf + 1) * 512],
                                psav[sl, :],
                                recz[hi][sl, mf * 512:(mf + 1) * 512], OP.mult)
                # output projection + bias + rowsum, DMA out
                for mt in range(MS // P):
                    mg = ms * (MS // P) + mt  # global m-chunk index
                    psw = ps_b.tile([P, 512], F32, tag="pb", name="pb")
                    for hc in range(HC):
                        nc.tensor.matmul(
                            psw, av_T[:, hc, mt * P:(mt + 1) * P],
                            wo_s[:, hc, :], start=(hc == 0), stop=(hc == HC - 1))
                    nc.vector.tensor_tensor(ww_tm[:, mg, :], psw, bo_b, OP.add)
                    nc.vector.tensor_reduce(
                        rs_cols[:, mg:mg + 1], ww_tm[:, mg, :], AX.X, OP.add)
                    if not DEBUG or mg >= 8:
                        nc.sync.dma_start(
                            ww_ext[mg * P:(mg + 1) * P, :], ww_tm[:, mg, :])

        s12.close()

        # ============ stage 3: argmax / threshold logic ============
        with ExitStack() as s3:
            pt = s3.enter_context(tc.tile_pool(name="tail", bufs=1))
            ptb = s3.enter_context(tc.tile_pool(name="tailb", bufs=3))
            ps_t = s3.enter_context(tc.tile_pool(name="ps_t", bufs=1,
                                                 space="PSUM"))
            ps_o = s3.enter_context(tc.tile_pool(name="ps_o", bufs=2,
                                                 space="PSUM"))
            # pos = argmax_m rowsum (first occurrence of max)
            iota_m = pt.tile([P, MC], I32, tag="iota_m")
            nc.gpsimd.iota(iota_m, pattern=[[P, MC]], base=0, channel_multiplier=1)
            iota_mf = pt.tile([P, MC], F32, tag="iota_mf")
            nc.any.tensor_copy(iota_mf, iota_m)
            rmax = pt.tile([P, 1], F32, tag="rmax")
            nc.vector.tensor_reduce(rmax, rs_cols, AX.X, OP.max)
            nc.gpsimd.partition_all_reduce(rmax, rmax, P, bass_isa.ReduceOp.max)
            eq = pt.tile([P, MC], F32, tag="eq")
            nc.vector.tensor_scalar(eq, rs_cols, rmax[:, 0:1], None, OP.is_equal)
            shift = pt.tile([P, MC], F32, tag="shift")
            nc.vector.tensor_scalar(shift, iota_mf, 3.0e6, None, OP.subtract)
            cand = pt.tile([P, MC], F32, tag="cand")
            nc.vector.tensor_tensor(cand, eq, shift, OP.mult)
            nc.vector.tensor_scalar(cand, cand, 3.0e6, None, OP.add)
            posn = pt.tile([P, 1], F32, tag="posn")
            nc.vector.tensor_reduce(posn, cand, AX.X, OP.min)
            nc.vector.tensor_scalar(posn, posn, -1.0, None, OP.mult)
            nc.gpsimd.partition_all_reduce(posn, posn, P, bass_isa.ReduceOp.max)
            pos_f = pt.tile([P, 1], F32, tag="pos_f")
            nc.vector.tensor_scalar(pos_f, posn, -1.0, None, OP.mult)
            oh_m = pt.tile([P, MC], F32, tag="oh_m")
            nc.vector.tensor_scalar(oh_m, iota_mf, pos_f[:, 0:1], None, OP.is_equal)

            # s_star = last token with imp > 0.5 ; exists = any
            iota_s = pt.tile([1, S], I32, tag="iota_s")
            nc.gpsimd.iota(iota_s, pattern=[[1, S]], base=0, channel_multiplier=0)
            iota_sf = pt.tile([1, S], F32, tag="iota_sf")
            nc.any.tensor_copy(iota_sf, iota_s)
            mask_s = pt.tile([1, S], F32, tag="mask_s")
            nc.vector.tensor_scalar(mask_s, imp_row, 0.5, None, OP.is_gt)
            exists = pt.tile([1, 1], F32, tag="exists")
            nc.vector.tensor_reduce(exists, mask_s, AX.X, OP.max)
            midx = pt.tile([1, S], F32, tag="midx")
            nc.vector.tensor_tensor(midx, mask_s, iota_sf, OP.mult)
            sstar = pt.tile([1, 1], F32, tag="sstar")
            nc.vector.tensor_reduce(sstar, midx, AX.X, OP.max)
            sstar_b = pt.tile([P, 1], F32, tag="sstar_b")
            nc.gpsimd.partition_broadcast(sstar_b, sstar)
            iota_sc2 = pt.tile([P, SC], I32, tag="iota_sc2")
            nc.gpsimd.iota(iota_sc2, pattern=[[P, SC]], base=0, channel_multiplier=1)
            iota_scf = pt.tile([P, SC], F32, tag="iota_scf")
            nc.any.tensor_copy(iota_scf, iota_sc2)
            oh_s = pt.tile([P, SC], F32, tag="oh_s")
            nc.vector.tensor_scalar(oh_s, iota_scf, sstar_b[:, 0:1], None, OP.is_equal)

            # ============ stage 4: gather, update MLP, scatter ============
            mb_tm = pt.tile([P, MC, D], F32, tag="mb_tm")
            nc.sync.dma_start(mb_tm[:], mb_ext.rearrange("(c p) d -> p c d", p=P))

            def gather_row(onehot, src, nch, tag):
                ps = ps_t.tile([P, 512], F32, tag="row", name="grow")
                for c in range(nch):
                    nc.tensor.matmul(ps[0:1, :], r(onehot[:, c:c + 1]),
                                     r(src[:, c, :]),
                                     start=(c == 0), stop=(c == nch - 1))
                row = pt.tile([1, 512], F32, tag=tag, name=tag)
                nc.vector.tensor_copy(row, ps[0:1, :])
                return row

            old_row = gather_row(oh_m, mb_tm, MC, "old_row")
            wwp_row = gather_row(oh_m, ww_tm, MC, "wwp_row")
            # sel: stream new_info token-major blocks
            ps_sel = ps_t.tile([P, 512], F32, tag="row2")
            for c in range(SC):
                nblk = ptb.tile([P, D], F32, tag="nblk", name="nblk")
                nc.sync.dma_start(nblk, ni_ext[c * P:(c + 1) * P, :])
                nc.tensor.matmul(ps_sel[0:1, :], r(oh_s[:, c:c + 1]), r(nblk),
                                 start=(c == 0), stop=(c == SC - 1))
            sel_row = pt.tile([1, 512], F32, tag="sel_row")
            nc.vector.tensor_copy(sel_row, ps_sel[0:1, :])

            # comb = [old | sel] as columns [128, 8]
            comb_col = pt.tile([P, 8], BF16, tag="comb_col")
            for half, row in enumerate((old_row, sel_row)):
                pad = pt.tile([P, 512], BF16, tag=f"pad{half}", name="pad")
                nc.vector.tensor_copy(pad, zero512)
                nc.vector.tensor_copy(pad[0:1, :], row)
                for c in range(4):
                    tps = ps_t.tile([P, P], BF16, tag="tp2", name="tp2")
                    nc.tensor.transpose(tps, pad[:, c * P:(c + 1) * P], ident_bf)
                    nc.vector.tensor_copy(
                        comb_col[:, half * 4 + c:half * 4 + c + 1],
                        tps[:, 0:1])

            # update MLP
            wu1 = pt.tile([P, 8, H], BF16, tag="wu1")
            nc.sync.dma_start(wu1[:], pext["Wu1"].rearrange("(c p) h -> p c h", p=P))
            bu1_c = pst.tile([P, HC], F32, tag="bu1_c")
            nc.sync.dma_start(bu1_c[:], pext["bu1"].rearrange("(c p) -> p c", p=P))
            wu2 = pt.tile([P, HC, H], BF16, tag="wu2")
            nc.sync.dma_start(wu2[:], pext["Wu2"].rearrange("(c p) h -> p c h", p=P))
            bu2_raw = pst.tile([1, H], F32, tag="bu2_raw")
            nc.sync.dma_start(bu2_raw[:], pext["bu2"].rearrange("(o h) -> o h", o=1))
            bu2_row = pst.tile([1, H], BF16, tag="bu2_row")
            nc.any.tensor_copy(bu2_row, bu2_raw)
            ones11 = pst.tile([1, 1], BF16, tag="ones11")
            nc.gpsimd.memset(ones11, 1.0)

            h1u_col = pt.tile([P, HC], BF16, tag="h1u_col")
            for hc in range(HC):
                ps = ps_t.tile([P, 512], F32, tag="row", name="mlprow")
                for kc in range(8):
                    nc.tensor.matmul(
                        ps[:, 0:1], r(wu1[:, kc, hc * P:(hc + 1) * P]),
                        r(comb_col[:, kc:kc + 1]),
                        start=(kc == 0), stop=(kc == 7))
                nc.scalar.activation(h1u_col[:, hc:hc + 1], ps[:, 0:1], AF.Relu,
                                     bias=bu1_c[:, hc:hc + 1])
            psu = ps_t.tile([P, 512], F32, tag="row3")
            for kc in range(HC):
                nc.tensor.matmul(psu[0:1, :], r(h1u_col[:, kc:kc + 1]),
                                 r(wu2[:, kc, :]), start=(kc == 0), stop=False)
            nc.tensor.matmul(psu[0:1, :], r(ones11), r(bu2_row),
                             start=False, stop=True)
            upd_row = pt.tile([1, H], F32, tag="upd_row")
            nc.scalar.activation(upd_row, psu[0:1, :], AF.Tanh, bias=0.0)

            delta_row = pt.tile([1, H], F32, tag="delta_row")
            nc.vector.tensor_tensor(delta_row, upd_row, wwp_row, OP.mult)
            nc.vector.tensor_scalar(delta_row, delta_row, exists[0:1, 0:1],
                                    None, OP.mult)
            delta_bf = pt.tile([1, H], BF16, tag="delta_bf")
            nc.any.tensor_copy(delta_bf, delta_row)

            # one-hot row for the rank-1 scatter
            ohpad = pt.tile([P, P], BF16, tag="ohpad")
            nc.vector.tensor_copy(ohpad, zero512[:, 0:P])
            nc.vector.tensor_copy(ohpad[:, 0:MC], oh_m)
            tpo = ps_t.tile([P, P], BF16, tag="tp2", name="tpo")
            nc.tensor.transpose(tpo, ohpad, ident_bf)
            oh_mT = pt.tile([MC, P], BF16, tag="oh_mT")
            nc.vector.tensor_copy(oh_mT, tpo[0:MC, :])
            oh_row = pt.tile([1, M], BF16, tag="oh_row")
            nc.sync.dma_start(oh_row.rearrange("o (c p) -> o c p", c=MC), oh_mT)

            for mc in range(MC):
                pso = ps_o.tile([P, 512], F32, tag="outer", name="outer")
                nc.tensor.matmul(pso, oh_row[0:1, mc * P:(mc + 1) * P],
                                 delta_bf, start=True, stop=True)
                ot = ptb.tile([P, D], F32, tag="ot", name="ot")
                nc.vector.tensor_tensor(ot, pso, mb_tm[:, mc, :], OP.add)
                nc.sync.dma_start(upd_ext[mc * P:(mc + 1) * P, :], ot)

    nc.compile()
    return nc


_NC = None


def _get_nc():
    global _NC
    if _NC is None:
        _NC = build()
    return _NC


def kernel(**inputs):
    nc = _get_nc()
    in_maps = []
    for i in range(B):
        m = {"new_info": np.ascontiguousarray(inputs["new_info"][i]),
             "memory_bank": np.ascontiguousarray(inputs["memory_bank"][i])}
        for k in PARAM_SPECS:
            a = np.asarray(inputs[k], np.float32)
            if k in MAT_PARAMS:
                a = a.astype(ml_dtypes.bfloat16)
            m[k] = np.ascontiguousarray(a)
        in_maps.append(m)
    res = run_bass_kernel_spmd(nc, in_maps, core_ids=list(range(B)))
    results = res.results
    updated = np.stack([results[i]["updated"] for i in range(B)])
    ww = np.stack([results[i]["write_weights"] for i in range(B)])
    imp = np.stack([results[i]["imp"] for i in range(B)])
    return updated, ww, imp
